# revision 20
# baseline (speedup 1.0000x reference)
"""Trainium2 kernel for out = A @ W2 @ B.T with banded Gaussian W2. (v4)

Math: W2 = W1*W1, W1[i,j] = exp(-(i-j)^2/128) truncated below 1e-10; W1 > eps
only for |i-j| <= 54, so in 128-blocks W2 is block-tridiagonal with three
distinct translation-invariant blocks (diag D0, super U, sub L = U.T).

Strategy (data-parallel over A's rows, 8 cores, no collectives):
  - host: transpose A and B once, quantize to bf16, and lay out every
    device tile partition-major so each DMA moves long contiguous
    per-partition lines (128 x 32KB segments for the B.T columns).
  - phase 1 (once per core): TT = W2 @ A.T over the [4096, 1024] slab via
    banded block-tridiag matmuls; TT stays resident in SBUF (bf16).
  - phase 2 (per 512-col chunk nu): out[:, nu] = TT.T @ B.T[:, nu]; the 8
    PSUM banks hold the 8 m-tiles while one 4MB column of B.T streams in
    per chunk (double-buffered, ~27 DMAs total per core).
  - all matmuls bf16 (1 cyc/row, fast weight load), fp32 PSUM accumulate;
    rel err ~5e-3, far inside the 2e-2 gate.
"""

import numpy as np
import ml_dtypes

import concourse.bass as bass
import concourse.mybir as mybir
from concourse import bacc
from concourse.bass_utils import run_bass_kernel_spmd
from concourse.tile import TileContext

P = 128          # partition / block size
N = 4096         # inner dims (A cols, B rows/cols)
M_FULL = 8192    # A rows
NCORES = 8
MS = M_FULL // NCORES   # 1024 rows of A per core
NK = N // P      # 32 contraction blocks
NM = MS // P     # 8 m-tiles per core
CW = 512         # output column chunk width (= 1 PSUM bank of fp32)
NCH = N // CW    # 8 chunks
NH = MS // CW    # 2 column-halves of the A.T slab in phase 1
AG = 4           # at blocks per DMA group
NAG = NK // AG   # 8 groups

SIGMA = 8.0
TRUNC_EPS = 1e-10

BF16 = np.dtype(ml_dtypes.bfloat16)

_COMPILED = {}


def _w2_block(dist):
    d = dist.astype(np.float32)
    w1 = np.exp(-(d * d) / np.float32(2.0 * SIGMA * SIGMA)).astype(np.float32)
    w1 = np.where(w1 > np.float32(TRUNC_EPS), w1, np.float32(0.0)).astype(np.float32)
    return (w1 * w1).astype(np.float32)


def _build_w2_pack():
    a = np.arange(P)[:, None]
    b = np.arange(P)[None, :]
    d0 = _w2_block(np.abs(a - b))          # W2[j, j]
    u = _w2_block(np.abs(a - b - P))       # W2[j-1, j]
    l = _w2_block(np.abs(P + a - b))       # W2[j+1, j]
    pack = np.concatenate([d0, u, l], axis=1)  # [128, 384]
    return np.ascontiguousarray(pack.astype(BF16))


def _build_program(reps=1, skip_phase1=False, skip_phase2=False,
                   skip_bt_dma=False, skip_mms=False, skip_copies=False):
    """Build + compile the Bass program (one NEFF, run SPMD on 8 cores).

    reps>1 repeats the computation serially inside the NEFF (timing
    calibration). The skip_* flags carve out pieces for timing
    decomposition only — they produce wrong results.
    """
    nc = bacc.Bacc("TRN2", target_bir_lowering=False, debug=False)
    f32 = mybir.dt.float32
    bf16 = mybir.dt.bfloat16

    # at[g, p, a*MS+f] = A.T[(g*AG+a)*128 + p, f]   (partition-major groups)
    at_dram = nc.dram_tensor(
        "at", [NAG, P, AG * MS], bf16, kind="ExternalInput"
    ).ap()
    # bt[nu, p, k*CW+f] = B.T[k*128+p, nu*512+f]    (partition-major columns)
    bt_dram = nc.dram_tensor(
        "bt", [NCH, P, NK * CW], bf16, kind="ExternalInput"
    ).ap()
    w2_dram = nc.dram_tensor("w2", [P, 3 * P], bf16, kind="ExternalInput").ap()
    # out[nu, p, m*CW+f] = out_slab[m*128+p, nu*512+f]
    out_dram = nc.dram_tensor(
        "out", [NCH, P, NM * CW], bf16, kind="ExternalOutput"
    ).ap()

    with TileContext(nc) as tc:
        with (
            tc.tile_pool(name="const", bufs=1) as const_pool,
            tc.tile_pool(name="atp", bufs=3) as at_pool,
            tc.tile_pool(name="ttp", bufs=1) as tt_pool,
            tc.tile_pool(name="btp", bufs=3) as bt_pool,
            tc.tile_pool(name="obp", bufs=2) as ob_pool,
            tc.tile_pool(name="psp", bufs=8, space="PSUM") as ps_pool,
        ):
            w2_sb = const_pool.tile([P, 3 * P], bf16, tag="w2", name="w2_sb")
            nc.sync.dma_start(w2_sb, w2_dram)
            w2_lhsT = {
                0: w2_sb[:, 0:P],
                -1: w2_sb[:, P:2 * P],
                1: w2_sb[:, 2 * P:3 * P],
            }

            for rep in range(reps):
                # --- phase 1: TT = W2 @ A.T ([4096, 1024] bf16 in SBUF)
                at_groups = [None] * NAG

                def get_at(k, rep=rep):
                    g = k // AG
                    if at_groups[g] is None:
                        at_t = at_pool.tile([P, AG * MS], bf16, tag="at",
                                            name=f"at_sb_{rep}_{g}")
                        eng = nc.sync if g % 2 == 0 else nc.scalar
                        eng.dma_start(at_t, at_dram[g])
                        at_groups[g] = at_t
                    kk = k % AG
                    return at_groups[g][:, kk * MS:(kk + 1) * MS]

                tt_tiles = []
                for j in range(NK):
                    tt_t = tt_pool.tile([P, MS], bf16, tag=f"tt{j}",
                                        name=f"tt_sb_{rep}_{j}")
                    if skip_phase1:
                        tt_tiles.append(tt_t)
                        continue
                    dlist = [d for d in (-1, 0, 1) if 0 <= j + d < NK]
                    for h in range(NH):
                        hs = bass.ts(h, CW)
                        ps_t = ps_pool.tile([P, CW], f32, tag="ps",
                                            name=f"ps_t_{rep}_{j}_{h}")
                        for i, d in enumerate(dlist):
                            nc.tensor.matmul(
                                ps_t,
                                lhsT=w2_lhsT[d],
                                rhs=get_at(j + d)[:, hs],
                                start=(i == 0),
                                stop=(i == len(dlist) - 1),
                            )
                        if (2 * j + h) % 2 == 0:
                            nc.vector.tensor_copy(tt_t[:, hs], ps_t)
                        else:
                            nc.scalar.copy(tt_t[:, hs], ps_t)
                    tt_tiles.append(tt_t)

                # --- phase 2: out = TT.T @ B.T, one 4MB bt column per nu,
                # loaded as two 2MB halves on the two parallel HWDGE rings.
                if skip_phase2:
                    continue
                bt_cols = [None] * NCH
                HB = NK * CW // 2  # half-column width

                def get_bt(nu, rep=rep):
                    if skip_bt_dma:
                        # timing variant: one shared resident column
                        if bt_cols[0] is None:
                            bt_t = bt_pool.tile([P, NK * CW], bf16, tag="bt",
                                                name=f"bt_sb_shared_{rep}")
                            nc.sync.dma_start(bt_t, bt_dram[0])
                            bt_cols[0] = bt_t
                        return bt_cols[0]
                    if bt_cols[nu] is None:
                        bt_t = bt_pool.tile([P, NK * CW], bf16, tag="bt",
                                            name=f"bt_sb_{rep}_{nu}")
                        # stripe the 4MB column across all three DMA issue
                        # paths (SP + ACT HWDGE rings, gpsimd SWDGE)
                        TH = NK * CW // 4
                        nc.sync.dma_start(bt_t[:, 0:TH],
                                          bt_dram[nu][:, 0:TH])
                        nc.scalar.dma_start(bt_t[:, TH:2 * TH],
                                            bt_dram[nu][:, TH:2 * TH])
                        nc.gpsimd.dma_start(bt_t[:, 2 * TH:3 * TH],
                                            bt_dram[nu][:, 2 * TH:3 * TH])
                        nc.sync.dma_start(bt_t[:, 3 * TH:4 * TH],
                                          bt_dram[nu][:, 3 * TH:4 * TH])
                        bt_cols[nu] = bt_t
                    return bt_cols[nu]

                get_bt(0)  # first column load overlaps phase 1

                get_bt(1)

                for nu in range(NCH):
                    bt_t = get_bt(nu)
                    for ahead in (1, 2):  # triple-buffer: prefetch 2 ahead
                        if nu + ahead < NCH:
                            get_bt(nu + ahead)
                    pure_dma = skip_mms and skip_phase1
                    ps_o = [
                        ps_pool.tile([P, CW], f32, tag="ps",
                                     name=f"ps_o_{rep}_{nu}_{m}")
                        for m in range(NM)
                    ] if not pure_dma else []
                    if not pure_dma:
                        klast = 0 if skip_mms else NK - 1
                        for k in range(klast + 1):
                            ks = bass.ts(k, CW)
                            for m in range(NM):
                                nc.tensor.matmul(
                                    ps_o[m],
                                    lhsT=tt_tiles[k][:, m * P:(m + 1) * P],
                                    rhs=bt_t[:, ks],
                                    start=(k == 0),
                                    stop=(k == klast),
                                )
                    ob_t = ob_pool.tile([P, NM * CW], bf16, tag="ob",
                                        name=f"ob_sb_{rep}_{nu}")
                    if not skip_copies:
                        for m in range(NM):
                            ms = bass.ts(m, CW)
                            if m % 2 == 0:
                                nc.vector.tensor_copy(ob_t[:, ms], ps_o[m])
                            else:
                                nc.scalar.copy(ob_t[:, ms], ps_o[m])
                    else:
                        nc.vector.memset(ob_t, 0)
                    nc.gpsimd.dma_start(out_dram[nu], ob_t)

    nc.compile()
    return nc


def _get_program():
    if "nc" not in _COMPILED:
        _COMPILED["nc"] = _build_program()
    return _COMPILED["nc"]


def _prep_inputs(A, B):
    """Host-side shard + quantize + retile (all partition-major)."""
    a_t = np.ascontiguousarray(A.T).astype(BF16)          # [4096, 8192]
    bt = np.ascontiguousarray(B.T).astype(BF16)           # [4096, 4096]
    # bt_h[nu, p, k*CW+f] = B.T[k*128+p, nu*512+f]
    bt_h = np.ascontiguousarray(
        bt.reshape(NK, P, NCH, CW).transpose(2, 1, 0, 3).reshape(NCH, P, NK * CW)
    )
    w2_pack = _build_w2_pack()
    maps = []
    for c in range(NCORES):
        slab = a_t[:, c * MS:(c + 1) * MS]                # [4096, 1024]
        at_h = np.ascontiguousarray(
            slab.reshape(NAG, AG, P, MS).transpose(0, 2, 1, 3)
            .reshape(NAG, P, AG * MS)
        )
        maps.append({"at": at_h, "bt": bt_h, "w2": w2_pack})
    return maps


def _untile_out(res):
    outs = []
    for c in range(NCORES):
        o = np.asarray(res.results[c]["out"])   # [NCH, P, NM*CW] bf16
        o = o.reshape(NCH, P, NM, CW).transpose(2, 1, 0, 3).reshape(MS, N)
        outs.append(o.astype(np.float32))
    return np.concatenate(outs, axis=0)


def kernel(A, B):
    A = np.ascontiguousarray(np.asarray(A, dtype=np.float32))
    B = np.ascontiguousarray(np.asarray(B, dtype=np.float32))
    assert A.shape == (M_FULL, N), A.shape
    assert B.shape == (N, N), B.shape

    in_maps = _prep_inputs(A, B)
    nc = _get_program()
    res = run_bass_kernel_spmd(nc, in_maps, core_ids=list(range(NCORES)))
    return _untile_out(res)


# revision 21
# speedup vs baseline: 1.3307x; 1.3307x over previous
"""Trainium2 kernel for out = A @ W2 @ B.T with banded Gaussian W2. (v4)

Math: W2 = W1*W1, W1[i,j] = exp(-(i-j)^2/128) truncated below 1e-10; W1 > eps
only for |i-j| <= 54, so in 128-blocks W2 is block-tridiagonal with three
distinct translation-invariant blocks (diag D0, super U, sub L = U.T).

Strategy (data-parallel over A's rows, 8 cores, no collectives):
  - host: transpose A and B once, quantize to bf16, and lay out every
    device tile partition-major so each DMA moves long contiguous
    per-partition lines (128 x 32KB segments for the B.T columns).
  - phase 1 (once per core): TT = W2 @ A.T over the [4096, 1024] slab via
    banded block-tridiag matmuls; TT stays resident in SBUF (bf16).
  - phase 2 (per 512-col chunk nu): out[:, nu] = TT.T @ B.T[:, nu]; the 8
    PSUM banks hold the 8 m-tiles while one 4MB column of B.T streams in
    per chunk (double-buffered, ~27 DMAs total per core).
  - all matmuls bf16 (1 cyc/row, fast weight load), fp32 PSUM accumulate;
    rel err ~5e-3, far inside the 2e-2 gate.
"""

import numpy as np
import ml_dtypes

import concourse.bass as bass
import concourse.mybir as mybir
from concourse import bacc
from concourse.bass_utils import run_bass_kernel_spmd
from concourse.tile import TileContext

P = 128          # partition / block size
N = 4096         # inner dims (A cols, B rows/cols)
M_FULL = 8192    # A rows
NCORES = 8
MS = M_FULL // NCORES   # 1024 rows of A per core
NK = N // P      # 32 contraction blocks
NM = MS // P     # 8 m-tiles per core
CW = 512         # output column chunk width (= 1 PSUM bank of fp32)
NCH = N // CW    # 8 chunks
NH = MS // CW    # 2 column-halves of the A.T slab in phase 1
AG = 4           # at blocks per DMA group
NAG = NK // AG   # 8 groups

SIGMA = 8.0
TRUNC_EPS = 1e-10

BF16 = np.dtype(ml_dtypes.bfloat16)

_COMPILED = {}


def _w2_block(dist):
    d = dist.astype(np.float32)
    w1 = np.exp(-(d * d) / np.float32(2.0 * SIGMA * SIGMA)).astype(np.float32)
    w1 = np.where(w1 > np.float32(TRUNC_EPS), w1, np.float32(0.0)).astype(np.float32)
    return (w1 * w1).astype(np.float32)


def _build_w2_pack():
    a = np.arange(P)[:, None]
    b = np.arange(P)[None, :]
    d0 = _w2_block(np.abs(a - b))          # W2[j, j]
    u = _w2_block(np.abs(a - b - P))       # W2[j-1, j]
    l = _w2_block(np.abs(P + a - b))       # W2[j+1, j]
    pack = np.concatenate([d0, u, l], axis=1)  # [128, 384]
    return np.ascontiguousarray(pack.astype(BF16))


def _build_program(reps=1, skip_phase1=False, skip_phase2=False,
                   skip_bt_dma=False, skip_mms=False, skip_copies=False):
    """Build + compile the Bass program (one NEFF, run SPMD on 8 cores).

    reps>1 repeats the computation serially inside the NEFF (timing
    calibration). The skip_* flags carve out pieces for timing
    decomposition only — they produce wrong results.
    """
    nc = bacc.Bacc("TRN2", target_bir_lowering=False, debug=False)
    f32 = mybir.dt.float32
    bf16 = mybir.dt.bfloat16

    # at[g, p, a*MS+f] = A.T[(g*AG+a)*128 + p, f]   (partition-major groups)
    at_dram = nc.dram_tensor(
        "at", [NAG, P, AG * MS], bf16, kind="ExternalInput"
    ).ap()
    # bt[nu, p, k*CW+f] = B.T[k*128+p, nu*512+f]    (partition-major columns)
    bt_dram = nc.dram_tensor(
        "bt", [NCH, P, NK * CW], bf16, kind="ExternalInput"
    ).ap()
    w2_dram = nc.dram_tensor("w2", [P, 3 * P], bf16, kind="ExternalInput").ap()
    # out[nu, p, m*CW+f] = out_slab[m*128+p, nu*512+f]
    out_dram = nc.dram_tensor(
        "out", [NCH, P, NM * CW], bf16, kind="ExternalOutput"
    ).ap()

    with TileContext(nc) as tc:
        with (
            tc.tile_pool(name="const", bufs=1) as const_pool,
            tc.tile_pool(name="atp", bufs=2) as at_pool,
            tc.tile_pool(name="ttp", bufs=1) as tt_pool,
            tc.tile_pool(name="btp", bufs=3) as bt_pool,
            tc.tile_pool(name="obp", bufs=2) as ob_pool,
            tc.tile_pool(name="psp", bufs=8, space="PSUM") as ps_pool,
        ):
            w2_sb = const_pool.tile([P, 3 * P], bf16, tag="w2", name="w2_sb")
            nc.sync.dma_start(w2_sb, w2_dram)
            w2_lhsT = {
                0: w2_sb[:, 0:P],
                -1: w2_sb[:, P:2 * P],
                1: w2_sb[:, 2 * P:3 * P],
            }

            for rep in range(reps):
                # --- phase 1: TT = W2 @ A.T ([4096, 1024] bf16 in SBUF)
                at_groups = [None] * NAG

                def get_at(k, rep=rep):
                    g = k // AG
                    if at_groups[g] is None:
                        at_t = at_pool.tile([P, AG * MS], bf16, tag="at",
                                            name=f"at_sb_{rep}_{g}")
                        eng = nc.sync if g % 2 == 0 else nc.scalar
                        eng.dma_start(at_t, at_dram[g])
                        at_groups[g] = at_t
                    kk = k % AG
                    return at_groups[g][:, kk * MS:(kk + 1) * MS]

                tt_tiles = []
                for j in range(NK):
                    tt_t = tt_pool.tile([P, MS], bf16, tag=f"tt{j}",
                                        name=f"tt_sb_{rep}_{j}")
                    if skip_phase1:
                        tt_tiles.append(tt_t)
                        continue
                    dlist = [d for d in (-1, 0, 1) if 0 <= j + d < NK]
                    for h in range(NH):
                        hs = bass.ts(h, CW)
                        ps_t = ps_pool.tile([P, CW], f32, tag="ps",
                                            name=f"ps_t_{rep}_{j}_{h}")
                        for i, d in enumerate(dlist):
                            nc.tensor.matmul(
                                ps_t,
                                lhsT=w2_lhsT[d],
                                rhs=get_at(j + d)[:, hs],
                                start=(i == 0),
                                stop=(i == len(dlist) - 1),
                            )
                        if (2 * j + h) % 2 == 0:
                            nc.vector.tensor_copy(tt_t[:, hs], ps_t)
                        else:
                            nc.scalar.copy(tt_t[:, hs], ps_t)
                    tt_tiles.append(tt_t)

                # --- phase 2: out = TT.T @ B.T, one 4MB bt column per nu,
                # loaded as two 2MB halves on the two parallel HWDGE rings.
                if skip_phase2:
                    continue
                bt_cols = [None] * NCH
                HB = NK * CW // 2  # half-column width

                def get_bt(nu, rep=rep):
                    if skip_bt_dma:
                        # timing variant: one shared resident column
                        if bt_cols[0] is None:
                            bt_t = bt_pool.tile([P, NK * CW], bf16, tag="bt",
                                                name=f"bt_sb_shared_{rep}")
                            nc.sync.dma_start(bt_t, bt_dram[0])
                            bt_cols[0] = bt_t
                        return bt_cols[0]
                    if bt_cols[nu] is None:
                        bt_t = bt_pool.tile([P, NK * CW], bf16, tag="bt",
                                            name=f"bt_sb_{rep}_{nu}")
                        # stripe the 4MB column across all three DMA issue
                        # paths (SP + ACT HWDGE rings, gpsimd SWDGE)
                        TH = NK * CW // 4
                        nc.sync.dma_start(bt_t[:, 0:TH],
                                          bt_dram[nu][:, 0:TH])
                        nc.scalar.dma_start(bt_t[:, TH:2 * TH],
                                            bt_dram[nu][:, TH:2 * TH])
                        nc.gpsimd.dma_start(bt_t[:, 2 * TH:3 * TH],
                                            bt_dram[nu][:, 2 * TH:3 * TH])
                        nc.sync.dma_start(bt_t[:, 3 * TH:4 * TH],
                                          bt_dram[nu][:, 3 * TH:4 * TH])
                        bt_cols[nu] = bt_t
                    return bt_cols[nu]

                get_bt(0)  # first column load overlaps phase 1

                get_bt(1)

                for nu in range(NCH):
                    bt_t = get_bt(nu)
                    for ahead in (1, 2):  # triple-buffer: prefetch 2 ahead
                        if nu + ahead < NCH:
                            get_bt(nu + ahead)
                    pure_dma = skip_mms and skip_phase1
                    ps_o = [
                        ps_pool.tile([P, CW], f32, tag="ps",
                                     name=f"ps_o_{rep}_{nu}_{m}")
                        for m in range(NM)
                    ] if not pure_dma else []
                    if not pure_dma:
                        klast = 0 if skip_mms else NK - 1
                        for k in range(klast + 1):
                            ks = bass.ts(k, CW)
                            for m in range(NM):
                                nc.tensor.matmul(
                                    ps_o[m],
                                    lhsT=tt_tiles[k][:, m * P:(m + 1) * P],
                                    rhs=bt_t[:, ks],
                                    start=(k == 0),
                                    stop=(k == klast),
                                )
                    ob_t = ob_pool.tile([P, NM * CW], bf16, tag="ob",
                                        name=f"ob_sb_{rep}_{nu}")
                    if not skip_copies:
                        for m in range(NM):
                            ms = bass.ts(m, CW)
                            if m % 2 == 0:
                                nc.vector.tensor_copy(ob_t[:, ms], ps_o[m])
                            else:
                                nc.scalar.copy(ob_t[:, ms], ps_o[m])
                    else:
                        nc.vector.memset(ob_t, 0)
                    nc.gpsimd.dma_start(out_dram[nu], ob_t)

    nc.compile()
    return nc


def _get_program():
    if "nc" not in _COMPILED:
        _COMPILED["nc"] = _build_program()
    return _COMPILED["nc"]


def _prep_inputs(A, B):
    """Host-side shard + quantize + retile (all partition-major)."""
    a_t = np.ascontiguousarray(A.T).astype(BF16)          # [4096, 8192]
    bt = np.ascontiguousarray(B.T).astype(BF16)           # [4096, 4096]
    # bt_h[nu, p, k*CW+f] = B.T[k*128+p, nu*512+f]
    bt_h = np.ascontiguousarray(
        bt.reshape(NK, P, NCH, CW).transpose(2, 1, 0, 3).reshape(NCH, P, NK * CW)
    )
    w2_pack = _build_w2_pack()
    maps = []
    for c in range(NCORES):
        slab = a_t[:, c * MS:(c + 1) * MS]                # [4096, 1024]
        at_h = np.ascontiguousarray(
            slab.reshape(NAG, AG, P, MS).transpose(0, 2, 1, 3)
            .reshape(NAG, P, AG * MS)
        )
        maps.append({"at": at_h, "bt": bt_h, "w2": w2_pack})
    return maps


def _untile_out(res):
    outs = []
    for c in range(NCORES):
        o = np.asarray(res.results[c]["out"])   # [NCH, P, NM*CW] bf16
        o = o.reshape(NCH, P, NM, CW).transpose(2, 1, 0, 3).reshape(MS, N)
        outs.append(o.astype(np.float32))
    return np.concatenate(outs, axis=0)


def kernel(A, B):
    A = np.ascontiguousarray(np.asarray(A, dtype=np.float32))
    B = np.ascontiguousarray(np.asarray(B, dtype=np.float32))
    assert A.shape == (M_FULL, N), A.shape
    assert B.shape == (N, N), B.shape

    in_maps = _prep_inputs(A, B)
    nc = _get_program()
    res = run_bass_kernel_spmd(nc, in_maps, core_ids=list(range(NCORES)))
    return _untile_out(res)
